# revision 19
# baseline (speedup 1.0000x reference)
"""MoELoRALinear Trainium2 kernel (8-core data-parallel, Bass/Tile).

Math (per token t, out feature o):
    out[t,o] = x[t,:] @ base_w[o,:] + base_b[o]
             + sum_e softmax_e(x[t,:] @ router_w[e,:]) * SCALE
               * sum_r (x[t,:] @ A[e,r,:]) * B[e,o,r]

Strategy (v2):
  - 8192 tokens sharded 8 ways (1024 tokens/core); weights replicated.
  - All matmul operands bf16 (error ~2e-3 « 2e-2 gate): halves DMA-in
    (24MB -> 12MB/core), faster startup ladder, less shared-HBM pressure.
    PSUM accumulation and softmax math stay fp32.
  - w1t ([A;router] table) loads FIRST on the sync ring: the PE's first
    instruction needs it, and the scalar ring kicks ~2us later.
  - Per 128-col K chunk: YT = [A;router].T-style matmul group (N=512)
    producing YT[36, 1024] (rank-proj rows 0:32, router logits 32:36),
    interleaved with wave-A base matmuls in DMA-arrival order.
  - Transposed softmax (no PE transposes, no per-t-chunk serial chains):
    logits are ~N(0,1) so exp() is overflow-safe without max-subtract.
    e4 = exp(logits) [4,1024]; one tiny matmul with a constant selection
    matrix replicates per-expert gates and the gate-sum to 32 partitions;
    two wide [32,1024] DVE multiplies produce the gated projection vw,
    cast to bf16 into the fused-accum lhsT (with a ones row for bias).
  - SCALE is folded into the B/bias table host-side.
  - Base matmul: per (oc,t) group 16 K-chunk matmuls (N=512) + one K=33
    close matmul ([vw;ones] @ [SCALE*B;bias]) accumulated into the same
    PSUM bank, then DVE copy to SBUF and DMA out.
"""

import os

import numpy as np

import concourse.bacc as bacc
import concourse.bass as bass
import concourse.mybir as mybir
from concourse.bass_utils import run_bass_kernel_spmd
from concourse.tile import TileContext

SCALE = 16.0 / 8.0  # alpha / r

N_CORES = 8
TOK = 8192  # 4 * 2048 tokens total
TPC = TOK // N_CORES  # tokens per core = 1024
D = 2048  # in features
O = 2048  # out features
E = 4
R = 8
ER = E * R  # 32
J = ER + E  # 36: rank-proj cols + router cols
DC = D // 128  # 16 contraction chunks
OCW = 512  # out-feature chunk width (one PSUM bank)
OC = O // OCW  # 4
TC = TPC // 128  # 8 token chunks per core

F32 = mybir.dt.float32
BF16 = mybir.dt.bfloat16

# Results of the last device run (for test harness inspection).
last_run_info: dict = {}

_cached = None


def _build_program():
    nc = bacc.Bacc()

    xt_d = nc.declare_dram_parameter("xt", [128, DC * TPC], BF16, isOutput=False)
    wt_d = nc.declare_dram_parameter("wt", [OC, 128, DC * OCW], BF16, isOutput=False)
    w1t_d = nc.declare_dram_parameter("w1t", [128, DC * J], BF16, isOutput=False)
    bcat_d = nc.declare_dram_parameter("bcat", [ER + 1, O], BF16, isOutput=False)
    sel_d = nc.declare_dram_parameter("sel", [E, 64], BF16, isOutput=False)
    out_d = nc.declare_dram_parameter("out", [OC, TC, 128, OCW], F32, isOutput=True)

    with TileContext(nc) as tc:
        with (
            tc.tile_pool(name="cpool", bufs=1) as cpool,
            tc.tile_pool(name="wpool", bufs=4) as wpool,
            tc.tile_pool(name="opool", bufs=6) as opool,
            tc.tile_pool(name="mpsum", bufs=6, space="PSUM") as mpsum,
            tc.tile_pool(name="ypsum", bufs=2, space="PSUM") as ypsum,
        ):
            # w1t first on the sync ring: it's the first thing the PE
            # needs.  bcat goes on the scalar ring (kicks ~2us later but
            # isn't needed until the first close, ~30us in).
            w1tr = cpool.tile([128, DC * J], BF16)
            nc.sync.dma_start(out=w1tr, in_=w1t_d[:, :])
            bcatr = cpool.tile([ER + 1, O], BF16)
            nc.scalar.dma_start(out=bcatr, in_=bcat_d[:, :])
            xtr = cpool.tile([128, DC * TPC], BF16)

            # Warm-up matmuls on scratch data, first in the Tensor queue
            # (and the scratch memset first in the Vector queue): the PE
            # p-state reaches max only after ~3us of continuous execution,
            # so spin it up during the DMA-kick window instead of paying
            # 2x-slow real matmuls once x arrives. Results land in a
            # scratch PSUM tile that is never read.
            scratch = cpool.tile([128, 256], BF16)
            nc.vector.memset(scratch, 0.0)
            # Shares the ypsum rotation: all warm matmuls retire before the
            # first YT matmul (same engine, in-order), so no added stall.
            warmps = ypsum.tile([128, 256], F32, name="warmps", tag="yb")
            for _ in range(10):
                nc.tensor.matmul(
                    warmps,
                    lhsT=scratch[:, 0:128],
                    rhs=scratch,
                    start=True,
                    stop=True,
                    skip_group_check=True,
                )

            def load_x(dc):
                nc.sync.dma_start(
                    out=xtr[:, dc * TPC : (dc + 1) * TPC],
                    in_=xt_d[:, dc * TPC : (dc + 1) * TPC],
                )

            def load_w(wtile, oc):
                for k in range(4):
                    nc.sync.dma_start(
                        out=wtile[:, k * 4 * OCW : (k + 1) * 4 * OCW],
                        in_=wt_d[oc, :, k * 4 * OCW : (k + 1) * 4 * OCW],
                    )

            # Interleave x-chunks with the matching 512KB wt[0] pieces so
            # wave-A base matmuls ladder along arriving data; wt0 piece k
            # goes right after the block's first x chunk so wave-A dc=4k
            # unlocks early. Remaining w tiles stream after x; all 4 stay
            # resident.
            wts = {oc: wpool.tile([128, DC * OCW], BF16, name=f"wtr{oc}", tag="wtr")
                   for oc in range(OC)}

            def load_w0_piece(k):
                nc.sync.dma_start(
                    out=wts[0][:, k * 4 * OCW : (k + 1) * 4 * OCW],
                    in_=wt_d[0, :, k * 4 * OCW : (k + 1) * 4 * OCW],
                )

            load_x(0)
            load_w0_piece(0)
            for dc in range(1, 4):
                load_x(dc)
            for k in range(1, 4):
                load_x(4 * k)
                load_x(4 * k + 1)
                load_w0_piece(k)
                load_x(4 * k + 2)
                load_x(4 * k + 3)
            for oc in range(1, OC):
                load_w(wts[oc], oc)

            # Constant selection matrix for the gate chain (host-built):
            #   sel[e, m] = 1 if m < 32 and m//8 == e   (gate replication)
            #   sel[e, m] = 1 if m >= 32                (gate-sum replication)
            selr = cpool.tile([E, 64], BF16)
            nc.scalar.dma_start(out=selr, in_=sel_d[:, :])

            # Gated projection lhsT + ones row (for bias), bf16.
            vwtr = cpool.tile([ER + 1, TPC], BF16)
            nc.vector.memset(vwtr[ER : ER + 1, :], 1.0)

            # --- Phase 1: YT (rank-proj + router logits, transposed) and
            # wave-A base matmuls, interleaved in DMA arrival order.
            ytps = [
                ypsum.tile([J, 512], F32, name=f"ytps{th}", tag="yb")
                for th in range(2)
            ]
            psA = {
                t: mpsum.tile([128, OCW], F32, name=f"ps0_{t}", tag="ps")
                for t in range(4)
            }
            # Per-dc interleave (2 YT + 4 wave-A matmuls ~= one x-chunk's
            # DMA time): the PE tracks arrival closely and YT's last chunk
            # retires right after x finishes, starting the gate chain as
            # early as possible.
            for dc in range(DC):
                for th in range(2):
                    nc.tensor.matmul(
                        ytps[th],
                        lhsT=w1tr[:, dc * J : (dc + 1) * J],
                        rhs=xtr[:, dc * TPC + th * 512 : dc * TPC + (th + 1) * 512],
                        start=(dc == 0),
                        stop=(dc == DC - 1),
                    )
                for t in range(4):
                    nc.tensor.matmul(
                        psA[t],
                        lhsT=xtr[:, dc * TPC + t * 128 : dc * TPC + (t + 1) * 128],
                        rhs=wts[0][:, dc * OCW : (dc + 1) * OCW],
                        start=(dc == 0),
                        stop=False,
                    )
            # Two more oc0 accumulation groups queued on the PE so it has
            # ~7us of independent work while the DVE/ACT gate chain runs.
            for t in (4, 5):
                psA[t] = mpsum.tile([128, OCW], F32, name=f"ps0_{t}", tag="ps")
                for dc in range(DC):
                    nc.tensor.matmul(
                        psA[t],
                        lhsT=xtr[:, dc * TPC + t * 128 : dc * TPC + (t + 1) * 128],
                        rhs=wts[0][:, dc * OCW : (dc + 1) * OCW],
                        start=(dc == 0),
                        stop=False,
                    )

            # --- Phase 2: gate chain, all-token-wide (no per-t loops).
            # Spread across engines for parallelism. GpSimd cannot touch
            # PSUM on TRN2, so it only gets the SBUF-only final multiply;
            # ACT (which can read PSUM) takes the th=1 copies as
            # Copy-activations; Vector does the rest (the custom-DVE approx
            # reciprocal is Vector-only, and mishandles PSUM input at
            # partition offset 32 — hence the SBUF staging of the gate-sum).
            Copy = mybir.ActivationFunctionType.Copy
            yt_sb = cpool.tile([J, TPC], F32)
            nc.vector.tensor_copy(yt_sb[:, 0:512], ytps[0])
            nc.scalar.activation(yt_sb[:, 512:1024], ytps[1], Copy)
            # exp(logits): logits ~ N(0,1), |l| < ~5.5, exp safe in fp32/bf16.
            e4 = cpool.tile([E, TPC], BF16)
            nc.scalar.activation(
                e4, yt_sb[ER:J, :], mybir.ActivationFunctionType.Exp
            )
            # Replicate gates (rows 0:32) and gate-sum (rows 32:64).
            gps = [
                ypsum.tile([64, 512], F32, name=f"gps{th}", tag="yb")
                for th in range(2)
            ]
            s_sb = cpool.tile([ER, TPC], F32)
            sinv = cpool.tile([ER, TPC], F32)
            ugat = cpool.tile([ER, TPC], F32)
            for th in range(2):
                sl = slice(th * 512, (th + 1) * 512)
                nc.tensor.matmul(
                    gps[th], lhsT=selr[:, 0:64], rhs=e4[:, sl], start=True, stop=True
                )
                if th == 0:
                    nc.vector.tensor_copy(s_sb[:, sl], gps[th][ER:64, :])
                else:
                    nc.scalar.activation(s_sb[:, sl], gps[th][ER:64, :], Copy)
                nc.vector.reciprocal_approx_fast(sinv[:, sl], s_sb[:, sl])
                nc.vector.tensor_mul(ugat[:, sl], yt_sb[0:ER, sl], gps[th][0:ER, :])
                if th == 0:
                    nc.gpsimd.tensor_mul(vwtr[0:ER, sl], ugat[:, sl], sinv[:, sl])
                else:
                    nc.vector.tensor_mul(vwtr[0:ER, sl], ugat[:, sl], sinv[:, sl])

            # --- Phase 3: base matmul + fused LoRA-up/bias accumulation
            def close_group(ps, oc, t, split=False):
                nc.tensor.matmul(
                    ps,
                    lhsT=vwtr[:, t * 128 : (t + 1) * 128],
                    rhs=bcatr[:, oc * OCW : (oc + 1) * OCW],
                    start=False,
                    stop=True,
                )
                ot = opool.tile([128, OCW], F32, tag="ot")
                if split:
                    # Last group: halve the copy/DMA so the final transfer
                    # pipelines instead of sitting wholly after the last MM.
                    for h in range(2):
                        hs = slice(h * (OCW // 2), (h + 1) * (OCW // 2))
                        nc.vector.tensor_copy(ot[:, hs], ps[:, hs])
                        nc.sync.dma_start(out=out_d[oc, t, :, hs], in_=ot[:, hs])
                else:
                    nc.vector.tensor_copy(ot, ps)
                    nc.sync.dma_start(out=out_d[oc, t], in_=ot)

            def full_group(wtr, oc, t, split=False):
                ps = mpsum.tile([128, OCW], F32, name=f"ps{oc}_{t}", tag="ps")
                for dc in range(DC):
                    nc.tensor.matmul(
                        ps,
                        lhsT=xtr[:, dc * TPC + t * 128 : dc * TPC + (t + 1) * 128],
                        rhs=wtr[:, dc * OCW : (dc + 1) * OCW],
                        start=(dc == 0),
                        stop=False,
                    )
                close_group(ps, oc, t, split=split)

            for t in range(6):
                close_group(psA[t], 0, t)
            for t in range(6, TC):
                full_group(wts[0], 0, t)
            del psA
            for oc in range(1, OC):
                for t in range(TC):
                    full_group(wts[oc], oc, t, split=(oc == OC - 1 and t == TC - 1))

    nc.compile()
    return nc


def _prep_inputs(x, base_w, base_b, A, B, router_w):
    """Host-side layout prep: per-partition-contiguous bf16 DMA images."""
    import ml_dtypes

    bf16 = ml_dtypes.bfloat16

    x2 = np.ascontiguousarray(x, dtype=np.float32).reshape(TOK, D)
    # xt[core][p, dc*TPC + t] = x2[core*TPC + t, dc*128 + p]
    xv = x2.reshape(N_CORES, TPC, DC, 128)
    xt = np.ascontiguousarray(xv.transpose(0, 3, 2, 1)).reshape(
        N_CORES, 128, DC * TPC
    ).astype(bf16)

    # wt[oc, p, dc*OCW + o] = base_w[oc*OCW + o, dc*128 + p]
    wv = np.ascontiguousarray(base_w, dtype=np.float32).reshape(OC, OCW, DC, 128)
    wt = np.ascontiguousarray(wv.transpose(0, 3, 2, 1)).reshape(
        OC, 128, DC * OCW
    ).astype(bf16)

    # W1 = [A flattened to 32 rows; router_w 4 rows] over D
    W1 = np.concatenate(
        [np.asarray(A, dtype=np.float32).reshape(ER, D), np.asarray(router_w, np.float32)],
        axis=0,
    )  # [36, D]
    w1v = W1.reshape(J, DC, 128)
    w1t = np.ascontiguousarray(w1v.transpose(2, 1, 0)).reshape(128, DC * J).astype(bf16)

    # bcat rows 0..31: SCALE * B[e, o, r] -> [er, o]; row 32: base_b  (bf16)
    bc = np.concatenate(
        [
            SCALE * np.asarray(B, dtype=np.float32).transpose(0, 2, 1).reshape(ER, O),
            np.asarray(base_b, dtype=np.float32)[None, :],
        ],
        axis=0,
    ).astype(bf16)  # [33, O]

    sel = np.zeros((E, 64), dtype=np.float32)
    for e in range(E):
        sel[e, e * R : (e + 1) * R] = 1.0
    sel[:, ER:64] = 1.0
    return xt, wt, w1t, bc, sel.astype(bf16)


def kernel(x, base_w, base_b, A, B, router_w):
    global _cached
    if _cached is None:
        _cached = _build_program()
    nc = _cached

    xt, wt, w1t, bc, sel = _prep_inputs(x, base_w, base_b, A, B, router_w)

    in_maps = [
        {"xt": xt[c], "wt": wt, "w1t": w1t, "bcat": bc, "sel": sel}
        for c in range(N_CORES)
    ]
    core_ids = list(range(N_CORES))

    profile = os.environ.get("KERNEL_PROFILE", "0") == "1"
    res = run_bass_kernel_spmd(nc, in_maps, core_ids, trace=profile)

    last_run_info.clear()
    last_run_info["exec_time_ns"] = res.exec_time_ns
    last_run_info["mean_exec_time_ns"] = res.mean_exec_time_ns
    last_run_info["instructions_and_trace"] = res.instructions_and_trace
    last_run_info["profile_json"] = res.profile_json

    # out[core] shape [OC, TC, 128, OCW] -> tokens x features
    full = np.empty((TOK, O), dtype=np.float32)
    for c in range(N_CORES):
        buf = res.results[c]["out"]  # [OC, TC, 128, OCW]
        full[c * TPC : (c + 1) * TPC] = (
            buf.transpose(1, 2, 0, 3).reshape(TPC, O)
        )
    return full.reshape(4, 2048, 2048)


# revision 21
# speedup vs baseline: 1.0341x; 1.0341x over previous
"""MoELoRALinear Trainium2 kernel (8-core data-parallel, Bass/Tile).

Math (per token t, out feature o):
    out[t,o] = x[t,:] @ base_w[o,:] + base_b[o]
             + sum_e softmax_e(x[t,:] @ router_w[e,:]) * SCALE
               * sum_r (x[t,:] @ A[e,r,:]) * B[e,o,r]

Strategy (v2):
  - 8192 tokens sharded 8 ways (1024 tokens/core); weights replicated.
  - All matmul operands bf16 (error ~2e-3 « 2e-2 gate): halves DMA-in
    (24MB -> 12MB/core), faster startup ladder, less shared-HBM pressure.
    PSUM accumulation and softmax math stay fp32.
  - w1t ([A;router] table) loads FIRST on the sync ring: the PE's first
    instruction needs it, and the scalar ring kicks ~2us later.
  - Per 128-col K chunk: YT = [A;router].T-style matmul group (N=512)
    producing YT[36, 1024] (rank-proj rows 0:32, router logits 32:36),
    interleaved with wave-A base matmuls in DMA-arrival order.
  - Transposed softmax (no PE transposes, no per-t-chunk serial chains):
    logits are ~N(0,1) so exp() is overflow-safe without max-subtract.
    e4 = exp(logits) [4,1024]; one tiny matmul with a constant selection
    matrix replicates per-expert gates and the gate-sum to 32 partitions;
    two wide [32,1024] DVE multiplies produce the gated projection vw,
    cast to bf16 into the fused-accum lhsT (with a ones row for bias).
  - SCALE is folded into the B/bias table host-side.
  - Base matmul: per (oc,t) group 16 K-chunk matmuls (N=512) + one K=33
    close matmul ([vw;ones] @ [SCALE*B;bias]) accumulated into the same
    PSUM bank, then DVE copy to SBUF and DMA out.
"""

import os

import numpy as np

import concourse.bacc as bacc
import concourse.bass as bass
import concourse.mybir as mybir
from concourse.bass_utils import run_bass_kernel_spmd
from concourse.tile import TileContext

SCALE = 16.0 / 8.0  # alpha / r

N_CORES = 8
TOK = 8192  # 4 * 2048 tokens total
TPC = TOK // N_CORES  # tokens per core = 1024
D = 2048  # in features
O = 2048  # out features
E = 4
R = 8
ER = E * R  # 32
J = ER + E  # 36: rank-proj cols + router cols
DC = D // 128  # 16 contraction chunks
OCW = 512  # out-feature chunk width (one PSUM bank)
OC = O // OCW  # 4
TC = TPC // 128  # 8 token chunks per core

F32 = mybir.dt.float32
BF16 = mybir.dt.bfloat16

# Results of the last device run (for test harness inspection).
last_run_info: dict = {}

_cached = None


def _build_program():
    nc = bacc.Bacc()

    xt_d = nc.declare_dram_parameter("xt", [128, DC * TPC], BF16, isOutput=False)
    wt_d = nc.declare_dram_parameter("wt", [OC, 128, DC * OCW], BF16, isOutput=False)
    w1t_d = nc.declare_dram_parameter("w1t", [128, DC * J], BF16, isOutput=False)
    bcat_d = nc.declare_dram_parameter("bcat", [ER + 1, O], BF16, isOutput=False)
    sel_d = nc.declare_dram_parameter("sel", [E, 64], BF16, isOutput=False)
    out_d = nc.declare_dram_parameter("out", [OC, TC, 128, OCW], F32, isOutput=True)

    with TileContext(nc) as tc:
        with (
            tc.tile_pool(name="cpool", bufs=1) as cpool,
            tc.tile_pool(name="wpool", bufs=4) as wpool,
            tc.tile_pool(name="opool", bufs=6) as opool,
            tc.tile_pool(name="mpsum", bufs=6, space="PSUM") as mpsum,
            tc.tile_pool(name="ypsum", bufs=2, space="PSUM") as ypsum,
        ):
            # w1t first on the sync ring: it's the first thing the PE
            # needs.  bcat goes on the scalar ring (kicks ~2us later but
            # isn't needed until the first close, ~30us in).
            w1tr = cpool.tile([128, DC * J], BF16)
            nc.sync.dma_start(out=w1tr, in_=w1t_d[:, :])
            bcatr = cpool.tile([ER + 1, O], BF16)
            nc.scalar.dma_start(out=bcatr, in_=bcat_d[:, :])
            xtr = cpool.tile([128, DC * TPC], BF16)

            # Warm-up matmuls on scratch data, first in the Tensor queue
            # (and the scratch memset first in the Vector queue): the PE
            # p-state reaches max only after ~3us of continuous execution,
            # so spin it up during the DMA-kick window instead of paying
            # 2x-slow real matmuls once x arrives. Results land in a
            # scratch PSUM tile that is never read.
            scratch = cpool.tile([128, 256], BF16)
            nc.vector.memset(scratch, 0.0)
            # Shares the ypsum rotation: all warm matmuls retire before the
            # first YT matmul (same engine, in-order), so no added stall.
            warmps = ypsum.tile([128, 256], F32, name="warmps", tag="yb")
            for _ in range(10):
                nc.tensor.matmul(
                    warmps,
                    lhsT=scratch[:, 0:128],
                    rhs=scratch,
                    start=True,
                    stop=True,
                    skip_group_check=True,
                )

            def load_x(dc):
                nc.sync.dma_start(
                    out=xtr[:, dc * TPC : (dc + 1) * TPC],
                    in_=xt_d[:, dc * TPC : (dc + 1) * TPC],
                )

            def load_w(wtile, oc):
                for k in range(4):
                    nc.sync.dma_start(
                        out=wtile[:, k * 4 * OCW : (k + 1) * 4 * OCW],
                        in_=wt_d[oc, :, k * 4 * OCW : (k + 1) * 4 * OCW],
                    )

            # Interleave x-chunks with the matching 512KB wt[0] pieces so
            # wave-A base matmuls ladder along arriving data; wt0 piece k
            # goes right after the block's first x chunk so wave-A dc=4k
            # unlocks early. Remaining w tiles stream after x; all 4 stay
            # resident.
            wts = {oc: wpool.tile([128, DC * OCW], BF16, name=f"wtr{oc}", tag="wtr")
                   for oc in range(OC)}

            def load_w0_piece(k):
                nc.sync.dma_start(
                    out=wts[0][:, k * 4 * OCW : (k + 1) * 4 * OCW],
                    in_=wt_d[0, :, k * 4 * OCW : (k + 1) * 4 * OCW],
                )

            # First block in half-granules: the PE (pre-warmed) outpaces the
            # DMA ladder at the start, so smaller first pieces unlock the
            # first real matmuls sooner.
            nc.sync.dma_start(out=xtr[:, 0:512], in_=xt_d[:, 0:512])
            nc.sync.dma_start(out=xtr[:, 512:1024], in_=xt_d[:, 512:1024])
            nc.sync.dma_start(
                out=wts[0][:, 0 : 2 * OCW], in_=wt_d[0, :, 0 : 2 * OCW]
            )
            load_x(1)
            nc.sync.dma_start(
                out=wts[0][:, 2 * OCW : 4 * OCW], in_=wt_d[0, :, 2 * OCW : 4 * OCW]
            )
            load_x(2)
            load_x(3)
            for k in range(1, 4):
                load_x(4 * k)
                load_x(4 * k + 1)
                load_w0_piece(k)
                load_x(4 * k + 2)
                load_x(4 * k + 3)
            for oc in range(1, OC):
                load_w(wts[oc], oc)

            # Constant selection matrix for the gate chain (host-built):
            #   sel[e, m] = 1 if m < 32 and m//8 == e   (gate replication)
            #   sel[e, m] = 1 if m >= 32                (gate-sum replication)
            selr = cpool.tile([E, 64], BF16)
            nc.scalar.dma_start(out=selr, in_=sel_d[:, :])

            # Gated projection lhsT + ones row (for bias), bf16.
            vwtr = cpool.tile([ER + 1, TPC], BF16)
            nc.vector.memset(vwtr[ER : ER + 1, :], 1.0)

            # --- Phase 1: YT (rank-proj + router logits, transposed) and
            # wave-A base matmuls, interleaved in DMA arrival order.
            ytps = [
                ypsum.tile([J, 512], F32, name=f"ytps{th}", tag="yb")
                for th in range(2)
            ]
            psA = {
                t: mpsum.tile([128, OCW], F32, name=f"ps0_{t}", tag="ps")
                for t in range(4)
            }
            # Per-dc interleave (2 YT + 4 wave-A matmuls ~= one x-chunk's
            # DMA time): the PE tracks arrival closely and YT's last chunk
            # retires right after x finishes, starting the gate chain as
            # early as possible.
            for dc in range(DC):
                for th in range(2):
                    nc.tensor.matmul(
                        ytps[th],
                        lhsT=w1tr[:, dc * J : (dc + 1) * J],
                        rhs=xtr[:, dc * TPC + th * 512 : dc * TPC + (th + 1) * 512],
                        start=(dc == 0),
                        stop=(dc == DC - 1),
                    )
                for t in range(4):
                    nc.tensor.matmul(
                        psA[t],
                        lhsT=xtr[:, dc * TPC + t * 128 : dc * TPC + (t + 1) * 128],
                        rhs=wts[0][:, dc * OCW : (dc + 1) * OCW],
                        start=(dc == 0),
                        stop=False,
                    )
            # --- Phase 2: gate chain, all-token-wide, interleaved with two
            # more oc0 accumulation groups (t4, t5) on the PE.
            #
            # Emission order matters: an engine op's semaphore wait anchors
            # to the LAST PE matmul emitted before it (program order), not
            # just its true data dependency. So the chain head goes right
            # after the per-dc loop (anchor = YT dc15), the gps matmuls sit
            # between the t4 and t5 groups (e4 is ready by then), and the
            # chain tail (anchor = gps) runs while the PE chews t5. The
            # closes then start with vwtr already complete — no PE stall.
            #
            # Engine split: GpSimd cannot touch PSUM on TRN2, so it only
            # gets an SBUF-only final multiply; ACT (which can read PSUM)
            # takes the th=1 copies as Copy-activations; Vector does the
            # rest (the custom-DVE approx reciprocal is Vector-only, and
            # mishandles PSUM input at partition offset 32 — hence the
            # SBUF staging of the gate-sum).
            Copy = mybir.ActivationFunctionType.Copy
            yt_sb = cpool.tile([J, TPC], F32)
            nc.vector.tensor_copy(yt_sb[:, 0:512], ytps[0])
            nc.scalar.activation(yt_sb[:, 512:1024], ytps[1], Copy)
            # exp(logits): logits ~ N(0,1), |l| < ~5.5, exp safe in fp32/bf16.
            e4 = cpool.tile([E, TPC], BF16)
            nc.scalar.activation(
                e4, yt_sb[ER:J, :], mybir.ActivationFunctionType.Exp
            )

            def accum_group(t):
                psA[t] = mpsum.tile([128, OCW], F32, name=f"ps0_{t}", tag="ps")
                for dc in range(DC):
                    nc.tensor.matmul(
                        psA[t],
                        lhsT=xtr[:, dc * TPC + t * 128 : dc * TPC + (t + 1) * 128],
                        rhs=wts[0][:, dc * OCW : (dc + 1) * OCW],
                        start=(dc == 0),
                        stop=False,
                    )

            accum_group(4)

            # Replicate gates (rows 0:32) and gate-sum (rows 32:64).
            gps = [
                ypsum.tile([64, 512], F32, name=f"gps{th}", tag="yb")
                for th in range(2)
            ]
            for th in range(2):
                sl = slice(th * 512, (th + 1) * 512)
                nc.tensor.matmul(
                    gps[th], lhsT=selr[:, 0:64], rhs=e4[:, sl], start=True, stop=True
                )
            s_sb = cpool.tile([ER, TPC], F32)
            sinv = cpool.tile([ER, TPC], F32)
            ugat = cpool.tile([ER, TPC], F32)
            for th in range(2):
                sl = slice(th * 512, (th + 1) * 512)
                if th == 0:
                    nc.vector.tensor_copy(s_sb[:, sl], gps[th][ER:64, :])
                else:
                    nc.scalar.activation(s_sb[:, sl], gps[th][ER:64, :], Copy)
                nc.vector.reciprocal_approx_fast(sinv[:, sl], s_sb[:, sl])
                nc.vector.tensor_mul(ugat[:, sl], yt_sb[0:ER, sl], gps[th][0:ER, :])
                if th == 0:
                    nc.gpsimd.tensor_mul(vwtr[0:ER, sl], ugat[:, sl], sinv[:, sl])
                else:
                    nc.vector.tensor_mul(vwtr[0:ER, sl], ugat[:, sl], sinv[:, sl])

            accum_group(5)

            # --- Phase 3: base matmul + fused LoRA-up/bias accumulation
            def close_group(ps, oc, t, split=False):
                nc.tensor.matmul(
                    ps,
                    lhsT=vwtr[:, t * 128 : (t + 1) * 128],
                    rhs=bcatr[:, oc * OCW : (oc + 1) * OCW],
                    start=False,
                    stop=True,
                )
                ot = opool.tile([128, OCW], F32, tag="ot")
                if split:
                    # Last group: halve the copy/DMA so the final transfer
                    # pipelines instead of sitting wholly after the last MM.
                    for h in range(2):
                        hs = slice(h * (OCW // 2), (h + 1) * (OCW // 2))
                        nc.vector.tensor_copy(ot[:, hs], ps[:, hs])
                        nc.sync.dma_start(out=out_d[oc, t, :, hs], in_=ot[:, hs])
                else:
                    nc.vector.tensor_copy(ot, ps)
                    nc.sync.dma_start(out=out_d[oc, t], in_=ot)

            def full_group(wtr, oc, t, split=False):
                ps = mpsum.tile([128, OCW], F32, name=f"ps{oc}_{t}", tag="ps")
                for dc in range(DC):
                    nc.tensor.matmul(
                        ps,
                        lhsT=xtr[:, dc * TPC + t * 128 : dc * TPC + (t + 1) * 128],
                        rhs=wtr[:, dc * OCW : (dc + 1) * OCW],
                        start=(dc == 0),
                        stop=False,
                    )
                close_group(ps, oc, t, split=split)

            for t in range(6):
                close_group(psA[t], 0, t)
            for t in range(6, TC):
                full_group(wts[0], 0, t)
            del psA
            for oc in range(1, OC):
                for t in range(TC):
                    full_group(wts[oc], oc, t, split=(oc == OC - 1 and t == TC - 1))

    nc.compile()
    return nc


def _prep_inputs(x, base_w, base_b, A, B, router_w):
    """Host-side layout prep: per-partition-contiguous bf16 DMA images."""
    import ml_dtypes

    bf16 = ml_dtypes.bfloat16

    x2 = np.ascontiguousarray(x, dtype=np.float32).reshape(TOK, D)
    # xt[core][p, dc*TPC + t] = x2[core*TPC + t, dc*128 + p]
    xv = x2.reshape(N_CORES, TPC, DC, 128)
    xt = np.ascontiguousarray(xv.transpose(0, 3, 2, 1)).reshape(
        N_CORES, 128, DC * TPC
    ).astype(bf16)

    # wt[oc, p, dc*OCW + o] = base_w[oc*OCW + o, dc*128 + p]
    wv = np.ascontiguousarray(base_w, dtype=np.float32).reshape(OC, OCW, DC, 128)
    wt = np.ascontiguousarray(wv.transpose(0, 3, 2, 1)).reshape(
        OC, 128, DC * OCW
    ).astype(bf16)

    # W1 = [A flattened to 32 rows; router_w 4 rows] over D
    W1 = np.concatenate(
        [np.asarray(A, dtype=np.float32).reshape(ER, D), np.asarray(router_w, np.float32)],
        axis=0,
    )  # [36, D]
    w1v = W1.reshape(J, DC, 128)
    w1t = np.ascontiguousarray(w1v.transpose(2, 1, 0)).reshape(128, DC * J).astype(bf16)

    # bcat rows 0..31: SCALE * B[e, o, r] -> [er, o]; row 32: base_b  (bf16)
    bc = np.concatenate(
        [
            SCALE * np.asarray(B, dtype=np.float32).transpose(0, 2, 1).reshape(ER, O),
            np.asarray(base_b, dtype=np.float32)[None, :],
        ],
        axis=0,
    ).astype(bf16)  # [33, O]

    sel = np.zeros((E, 64), dtype=np.float32)
    for e in range(E):
        sel[e, e * R : (e + 1) * R] = 1.0
    sel[:, ER:64] = 1.0
    return xt, wt, w1t, bc, sel.astype(bf16)


def kernel(x, base_w, base_b, A, B, router_w):
    global _cached
    if _cached is None:
        _cached = _build_program()
    nc = _cached

    xt, wt, w1t, bc, sel = _prep_inputs(x, base_w, base_b, A, B, router_w)

    in_maps = [
        {"xt": xt[c], "wt": wt, "w1t": w1t, "bcat": bc, "sel": sel}
        for c in range(N_CORES)
    ]
    core_ids = list(range(N_CORES))

    profile = os.environ.get("KERNEL_PROFILE", "0") == "1"
    res = run_bass_kernel_spmd(nc, in_maps, core_ids, trace=profile)

    last_run_info.clear()
    last_run_info["exec_time_ns"] = res.exec_time_ns
    last_run_info["mean_exec_time_ns"] = res.mean_exec_time_ns
    last_run_info["instructions_and_trace"] = res.instructions_and_trace
    last_run_info["profile_json"] = res.profile_json

    # out[core] shape [OC, TC, 128, OCW] -> tokens x features
    full = np.empty((TOK, O), dtype=np.float32)
    for c in range(N_CORES):
        buf = res.results[c]["out"]  # [OC, TC, 128, OCW]
        full[c * TPC : (c + 1) * TPC] = (
            buf.transpose(1, 2, 0, 3).reshape(TPC, O)
        )
    return full.reshape(4, 2048, 2048)
